# revision 9
# baseline (speedup 1.0000x reference)
"""Distributed scatter-max (segment max over edge targets) on 8 TRN2 NeuronCores.

Strategy (segment-parallel scatter per the sharding hint):
  * Host shuffles edges: sort by target node; group nodes by exact degree and
    deal each degree class round-robin into 32 streams (8 cores x 4 lanes), so
    every stream has the IDENTICAL sequence of (window, count) chunks -> one
    SPMD graph serves all cores. Each node's edges are contiguous in its
    stream; streams are laid out feature-major (D=32 dims on SBUF partitions,
    edge slots along the free axis), cast to fp16 (max is order-exact; fp16
    rounding keeps rel err ~3e-4, far under the 2e-2 gate).
  * Device (same graph on all 8 cores) streams its (128, Q) slab tile by tile
    on the sync HWDGE queue and runs one windowed max tensor_reduce per degree
    chunk (window = exact degree -> zero compute waste), writing per-node
    columns; per-tile output slices leave on the scalar HWDGE queue so output
    never blocks the input stream.
  * Host gathers each node's column, casts back to f32; empty nodes stay 0.
"""
import sys

import numpy as np

try:
    import concourse.bacc as bacc
except ImportError:
    sys.path.insert(0, "/opt/trn_rl_repo")
    import concourse.bacc as bacc

import concourse.tile as tile
from concourse import mybir
from concourse.bass_utils import run_bass_kernel_spmd

C = 8            # cores
L = 4            # lanes per core (L * D = 128 partitions)
D = 32           # feature dim
P = 128
NLANES = C * L
T_MAX = 4096     # max edge slots per tile
FILL = -60000.0  # pad value (below any fp16 data value we handle)
DT = mybir.dt.float16

_graph_cache = {}


def _plan(tgt, N):
    """Host-side shuffle plan. Returns layout dict (see kernel())."""
    E = tgt.shape[0]
    deg = np.bincount(tgt, minlength=N).astype(np.int64)

    nz = np.flatnonzero(deg > 0)
    order_by_deg = nz[np.argsort(deg[nz], kind="stable")]
    degs_sorted = deg[order_by_deg]
    uniq, counts = np.unique(degs_sorted, return_counts=True)

    # DP: partition the sorted distinct degrees into contiguous groups
    # (window = group max degree) minimizing total slots incl. lane-dealing
    # dummies. Window-limited for pathological degree spreads.
    K = len(uniq)
    W_DP = 48
    best = [0.0] * (K + 1)
    choice = [0] * (K + 1)
    for j in range(1, K + 1):
        b, bi = None, 0
        for i in range(max(0, j - W_DP), j):
            c = int(counts[i:j].sum())
            npl = -(-c // NLANES)
            v = best[i] + npl * NLANES * int(uniq[j - 1])
            if b is None or v < b:
                b, bi = v, i
        best[j], choice[j] = b, bi
    cuts = []
    j = K
    while j > 0:
        i = choice[j]
        cuts.append((i, j))
        j = i
    cuts.reverse()

    node_lane = np.full(N, -1, dtype=np.int32)
    node_rank = np.full(N, -1, dtype=np.int64)   # per-lane sequence index
    classes = []          # (window s, nodes per lane)
    seq_len = 0
    base = 0
    for i, j in cuts:
        cnt = int(counts[i:j].sum())
        s = int(uniq[j - 1])
        ids = order_by_deg[base:base + cnt]
        base += cnt
        npl = (cnt + NLANES - 1) // NLANES
        node_lane[ids] = np.arange(cnt, dtype=np.int32) % NLANES
        node_rank[ids] = seq_len + np.arange(cnt, dtype=np.int64) // NLANES
        classes.append((s, int(npl)))
        seq_len += npl

    needed0 = sum(s * npl for s, npl in classes)
    # a node's window must fit inside one tile
    max_s = max(s for s, _ in classes)
    t_max = max(T_MAX, ((max_s + 63) // 64) * 64)
    assert max_s <= 24576, f"node degree {max_s} exceeds supported maximum"

    def walk(bounds):
        chunks = []           # (tile, off, n, s, col0)
        node_pos_seq = np.empty(seq_len, dtype=np.int64)
        ti = pos = col = seq_base = 0
        for s, npl in classes:
            remaining = npl
            while remaining > 0:
                if ti >= len(bounds):
                    return None
                start, size = bounds[ti]
                space = start + size - pos
                fit = min(remaining, space // s)
                if fit == 0:
                    pos = start + size
                    ti += 1
                    continue
                chunks.append((ti, int(pos - start), int(fit), int(s), int(col)))
                idx0 = seq_base + (npl - remaining)
                node_pos_seq[idx0:idx0 + fit] = pos + np.arange(fit, dtype=np.int64) * s
                pos += fit * s
                col += fit
                remaining -= fit
            seq_base += npl
        return chunks, node_pos_seq, col

    slack = 0
    while True:
        rem = needed0 + slack
        sizes = []
        # small head tiles so the vector engine starts early
        for t in (1024, 3072):
            if rem > 2 * t_max and t > max_s:
                sizes.append(t)
                rem -= t
        while rem > t_max:
            sizes.append(t_max)
            rem -= t_max
        sizes.append(((rem + 63) // 64) * 64)
        bounds = []
        acc = 0
        for t in sizes:
            bounds.append((acc, t))
            acc += t
        r = walk(bounds)
        if r is not None:
            break
        slack += 256
    chunks, node_pos_seq, NN = r

    node_pos = np.zeros(N, dtype=np.int64)
    node_col = np.full(N, -1, dtype=np.int64)
    m = node_rank >= 0
    node_pos[m] = node_pos_seq[node_rank[m]]
    node_col[m] = node_rank[m]           # cols assigned in walk (= seq) order

    order = np.argsort(tgt, kind="stable")
    sorted_tgt = tgt[order]
    starts = np.searchsorted(sorted_tgt, np.arange(N), side="left")
    rank = np.arange(E, dtype=np.int64) - starts[sorted_tgt]
    slot = node_pos[sorted_tgt] + rank
    elane = node_lane[sorted_tgt]

    return dict(chunks=chunks, tile_bounds=bounds, Q=int(acc), NN=int(NN),
                node_lane=node_lane, node_col=node_col, nz=deg > 0,
                order=order, slot=slot, elane=elane)


def _build_graph(ly):
    key = (ly["Q"], ly["NN"], tuple(ly["tile_bounds"]), tuple(ly["chunks"]))
    if key in _graph_cache:
        return _graph_cache[key]
    Q, NN = ly["Q"], ly["NN"]
    tile_bounds = ly["tile_bounds"]
    NT = len(tile_bounds)
    nc = bacc.Bacc()
    x_ext = nc.declare_dram_parameter("xt", [P, Q], DT, isOutput=False)
    out_ext = nc.declare_dram_parameter("out", [P, NN], DT, isOutput=True)
    by_tile = [[] for _ in range(NT)]
    for (ti, off, n, s, col0) in ly["chunks"]:
        by_tile[ti].append((off, n, s, col0))
    tile_cols = []
    for i in range(NT):
        cs = by_tile[i]
        c0 = min(c[3] for c in cs) if cs else 0
        c1 = max(c[3] + c[1] for c in cs) if cs else 0
        tile_cols.append((c0, c1))

    with tile.TileContext(nc) as tc:
        with tc.tile_pool(name="x", bufs=3) as xp, \
             tc.tile_pool(name="o", bufs=3) as opool, \
             tc.tile_pool(name="g", bufs=3) as gpool:
            for i in range(NT):
                start, size = tile_bounds[i]
                xt = xp.tile([P, size], DT, tag="xt")
                ieng = nc.scalar if i % 2 else nc.sync
                ieng.dma_start(out=xt[:], in_=x_ext[:, start:start + size])
                c0, c1 = tile_cols[i]
                if c1 <= c0:
                    continue
                ot = opool.tile([P, c1 - c0], DT, tag="ot")
                for (off, n, s, col0) in by_tile[i]:
                    if s == 1:
                        nc.vector.tensor_copy(ot[:, col0 - c0:col0 - c0 + n],
                                              xt[:, off:off + n])
                    elif s >= 6:
                        # pairwise-max levels via tensor_tensor (streams 2
                        # inputs/cycle, ~2.6x faster than reduce per input
                        # elem); overlapping halves are fine: max is idempotent
                        h = (s + 1) // 2
                        view = xt[:, off:off + n * s].rearrange(
                            "p (n s) -> p n s", s=s)
                        tmp = gpool.tile([P, n * h], DT, tag="gp")
                        tv = tmp[:].rearrange("p (n h) -> p n h", h=h)
                        nc.vector.tensor_tensor(tv, view[:, :, 0:h],
                                                view[:, :, s - h:s],
                                                mybir.AluOpType.max)
                        if s >= 12:
                            h2 = (h + 1) // 2
                            tmp2 = gpool.tile([P, n * h2], DT, tag="gp2")
                            tv2 = tmp2[:].rearrange("p (n h) -> p n h", h=h2)
                            nc.vector.tensor_tensor(tv2, tv[:, :, 0:h2],
                                                    tv[:, :, h - h2:h],
                                                    mybir.AluOpType.max)
                            tv = tv2
                        nc.vector.tensor_reduce(
                            out=ot[:, col0 - c0:col0 - c0 + n],
                            in_=tv,
                            axis=mybir.AxisListType.X,
                            op=mybir.AluOpType.max,
                        )
                    else:
                        nc.vector.tensor_reduce(
                            out=ot[:, col0 - c0:col0 - c0 + n],
                            in_=xt[:, off:off + n * s].rearrange(
                                "p (n s) -> p n s", s=s),
                            axis=mybir.AxisListType.X,
                            op=mybir.AluOpType.max,
                        )
                nc.scalar.dma_start(out=out_ext[:, c0:c1], in_=ot[:])
    nc.finalize()
    _graph_cache[key] = nc
    return nc


def kernel(source_node_representation_with_coefficient, edge_index, num_nodes):
    x = np.asarray(source_node_representation_with_coefficient, dtype=np.float32)
    tgt = np.asarray(edge_index)[1].astype(np.int64)
    N = int(num_nodes)
    E, d = x.shape
    assert d == D, f"kernel hardcodes D={D}, got {d}"
    if E == 0 or N == 0:
        return np.zeros((N, D), dtype=np.float32)

    ly = _plan(tgt, N)
    Q = ly["Q"]

    # scatter fp16-cast edge features into the padded lane layout
    x16 = np.clip(x, -60000.0, 60000.0).astype(np.float16)
    perm = np.full((NLANES, Q), E, dtype=np.int64)
    perm[ly["elane"], ly["slot"]] = ly["order"]
    x_aug = np.concatenate(
        [x16, np.full((1, D), FILL, dtype=np.float16)], axis=0)
    g = x_aug[perm]                                   # (32, Q, D)
    g = g.reshape(C, L, Q, D).transpose(0, 1, 3, 2)   # (C, L, D, Q)
    xt_all = np.ascontiguousarray(g.reshape(C, P, Q))

    nc = _build_graph(ly)
    in_maps = [{"xt": xt_all[c]} for c in range(C)]
    res = run_bass_kernel_spmd(nc, in_maps, core_ids=list(range(C)))

    v = np.stack([res.results[c]["out"] for c in range(C)])   # (C, P, NN) f16
    out = np.zeros((N, D), dtype=np.float32)
    nzi = np.flatnonzero(ly["nz"])
    gl = ly["node_lane"][nzi].astype(np.int64)
    core, lane = gl // L, gl % L
    colv = ly["node_col"][nzi]
    rows = (lane * D)[:, None] + np.arange(D)[None, :]
    out[nzi] = v[core[:, None], rows, colv[:, None]].astype(np.float32)
    return out
